# revision 7
# baseline (speedup 1.0000x reference)
"""Patch-local cross attention (CSA) TRN2 kernel.

Problem (hardcoded shapes): B=32, C=512, lohw=56, hihw=28.
base = hihw//7 = 4, rate = (lohw//hihw)*base = 8.
lo_p: [B, 49, 64, C], hi_p: [B, 49, 16, C] (7x7 patch grid).
q = lo_p@WqT+bq; k = hi_p@WkT+bk; v = hi_p@WvT+bv
gate = gelu(lo_p@Ws.T+bs)  (exact erf gelu)
out = softmax(q k^T / sqrt(C)) @ v * gate + lo_p, unpatched.

Sharding: data-parallel over B across 8 cores (4 batch items each).
Host does patching/transposes/bf16 casts and the final fp32 residual
add (+lo_p) + unpatch; device does all matmuls, softmax, gelu, scaling.

Device layout per core (NB=4 batch items):
  loT  [C=512, NB*3136] bf16 (c-major, patch-major token order)
  hiT  [C=512, NB*784]  bf16
  per batch item b: K^T, V resident in SBUF; lo processed in 7
  patch-tiles (7 patches = 448 q-tokens, 112 kv-tokens).
  scores^T computed densely [112, 448] per patch-tile, exp'd,
  block-diag masked; unnormalized attn@v in 2-patch groups (K=32);
  final per-token scale = gelu(gate)/denom (per-partition scalar).
Output: gated attention term [NB*3136, 512] fp32 (residual on host).
"""

import sys

if "/opt/trn_rl_repo" not in sys.path:
    sys.path.insert(0, "/opt/trn_rl_repo")

import numpy as np
import ml_dtypes

import concourse.bacc as bacc
import concourse.bass as bass
import concourse.mybir as mybir
from concourse import tile
from concourse.bass_utils import run_bass_kernel_spmd

BF16 = mybir.dt.bfloat16
F32 = mybir.dt.float32
NPBF16 = ml_dtypes.bfloat16

N_CORES = 8
B, C, LOHW, HIHW = 32, 512, 56, 28
RATE, BASE = 8, 4
G = 7               # patch grid side
P = G * G           # 49 patches
NQ = RATE * RATE    # 64 q tokens / patch
NK = BASE * BASE    # 16 kv tokens / patch
NB = B // N_CORES   # batch items per core
TLO = NB * P * NQ   # 12544 lo tokens per core
THI = NB * P * NK   # 3136 hi tokens per core
NCH = C // 128      # 4 contraction chunks
PT = 7              # patches per tile
QW = PT * NQ        # 448 q tokens per patch-tile
KW = PT * NK        # 112 kv tokens per patch-tile
NPT = P // PT       # 7 patch-tiles per batch item
SCALE = float(C) ** -0.5


def build_program(n_b: int = NB, gate_act: str = "gelu", repeat: int = 1):
    """Emit the Bass/Tile program for one core processing n_b batch items.

    gate_act="identity" substitutes the gate's Gelu with Identity so the
    program can run under CoreSim (which lacks a Gelu model)."""
    act_fn = (
        mybir.ActivationFunctionType.Gelu
        if gate_act == "gelu"
        else mybir.ActivationFunctionType.Identity
    )
    nc = bacc.Bacc(
        "TRN2",
        target_bir_lowering=False,
        debug=False,
        num_devices=N_CORES,
    )
    tlo = n_b * P * NQ
    thi = n_b * P * NK

    loT_d = nc.dram_tensor("loT", [C, tlo], BF16, kind="ExternalInput").ap()
    hiT_d = nc.dram_tensor("hiT", [C, thi], BF16, kind="ExternalInput").ap()
    wqT_d = nc.dram_tensor("wqT", [C, C], BF16, kind="ExternalInput").ap()
    wkT_d = nc.dram_tensor("wkT", [C, C], BF16, kind="ExternalInput").ap()
    wvT_d = nc.dram_tensor("wvT", [C, C], BF16, kind="ExternalInput").ap()
    ws_d = nc.dram_tensor("ws", [128, NCH], F32, kind="ExternalInput").ap()
    bq_d = nc.dram_tensor("bq", [128, NCH], F32, kind="ExternalInput").ap()
    bk_d = nc.dram_tensor("bk", [128, NCH], F32, kind="ExternalInput").ap()
    bv_d = nc.dram_tensor("bv", [1, C], BF16, kind="ExternalInput").ap()
    bs_d = nc.dram_tensor("bs", [128, 1], F32, kind="ExternalInput").ap()
    mask_d = nc.dram_tensor("mask", [KW, QW], BF16, kind="ExternalInput").ap()
    out_d = nc.dram_tensor("out", [tlo, C], F32, kind="ExternalOutput").ap()

    with tile.TileContext(nc) as tc:
        with (
            tc.tile_pool(name="const", bufs=1) as cpool,
            tc.tile_pool(name="kv", bufs=2) as kvpool,
            tc.tile_pool(name="work", bufs=2) as wpool,
            tc.tile_pool(name="lo", bufs=2) as lopool,
            tc.tile_pool(name="aout", bufs=3) as apool,
            tc.tile_pool(name="pproj", bufs=2, space=bass.MemorySpace.PSUM) as pproj,
            tc.tile_pool(name="pvao", bufs=2, space=bass.MemorySpace.PSUM) as pvao,
            tc.tile_pool(name="psc", bufs=2, space=bass.MemorySpace.PSUM) as psc,
            tc.tile_pool(name="pgd", bufs=2, space=bass.MemorySpace.PSUM) as pgd,
        ):
            # ---- constants ----
            wq_sb = [cpool.tile([128, C], BF16, tag=f"wq{j}", name=f"wq{j}") for j in range(NCH)]
            wk_sb = [cpool.tile([128, C], BF16, tag=f"wk{j}", name=f"wk{j}") for j in range(NCH)]
            wv_sb = [cpool.tile([128, C], BF16, tag=f"wv{j}", name=f"wv{j}") for j in range(NCH)]
            for j in range(NCH):
                nc.sync.dma_start(wq_sb[j][:], wqT_d[128 * j : 128 * (j + 1), :])
                nc.sync.dma_start(wk_sb[j][:], wkT_d[128 * j : 128 * (j + 1), :])
                nc.sync.dma_start(wv_sb[j][:], wvT_d[128 * j : 128 * (j + 1), :])
            ws_sb = cpool.tile([128, NCH], F32, tag="ws", name="ws")
            bq_sb = cpool.tile([128, NCH], F32, tag="bq", name="bq")
            bk_sb = cpool.tile([128, NCH], F32, tag="bk", name="bk")
            bv_sb = cpool.tile([1, C], BF16, tag="bv", name="bv")
            bs_sb = cpool.tile([128, 1], F32, tag="bs", name="bs")
            mask_sb = cpool.tile([KW, QW], BF16, tag="mask", name="mask")
            nc.sync.dma_start(ws_sb[:], ws_d[:])
            nc.sync.dma_start(bq_sb[:], bq_d[:])
            nc.sync.dma_start(bk_sb[:], bk_d[:])
            nc.sync.dma_start(bv_sb[:], bv_d[:])
            nc.sync.dma_start(bs_sb[:], bs_d[:])
            nc.sync.dma_start(mask_sb[:], mask_d[:])
            ones_col = cpool.tile([128, 1], BF16, tag="ones_col", name="ones_col")
            ones_row = cpool.tile([1, KW], BF16, tag="ones_row", name="ones_row")
            nc.vector.memset(ones_col[:], 1.0)
            nc.vector.memset(ones_row[:], 1.0)

            def body():
              for b in range(n_b):
                # ---- K/V phase: project all of hi[b] ----
                hiT_sb = [
                    kvpool.tile([128, P * NK], BF16, tag=f"hiT{j}", name=f"hiT{j}")
                    for j in range(NCH)
                ]
                for j in range(NCH):
                    nc.sync.dma_start(
                        hiT_sb[j][:],
                        hiT_d[
                            128 * j : 128 * (j + 1),
                            b * P * NK : (b + 1) * P * NK,
                        ],
                    )
                # K^T [c, kv_tok] resident, by c-chunk
                kT_sb = [
                    kvpool.tile([128, P * NK], BF16, tag=f"kT{j}", name=f"kT{j}")
                    for j in range(NCH)
                ]
                halves = [(0, 448), (448, P * NK)]
                for m in range(NCH):
                    for h0, h1 in halves:
                        ps = pproj.tile([128, QW], F32, tag="proj", name="proj")
                        for j in range(NCH):
                            nc.tensor.matmul(
                                ps[:, : h1 - h0],
                                wk_sb[j][:, 128 * m : 128 * (m + 1)],
                                hiT_sb[j][:, h0:h1],
                                start=(j == 0),
                                stop=(j == NCH - 1),
                            )
                        nc.vector.tensor_scalar_add(
                            kT_sb[m][:, h0:h1],
                            ps[:, : h1 - h0],
                            bk_sb[:, m : m + 1],
                        )
                # V [kv_tok, c] resident (token-major), by patch-tile
                v_sb = kvpool.tile([KW, NPT * C], BF16, tag="v", name="v")
                for pt in range(NPT):
                    w0 = pt * KW
                    ps = pvao.tile([128, C], F32, tag="vao", name="vao")
                    for j in range(NCH):
                        nc.tensor.matmul(
                            ps[:KW, :],
                            hiT_sb[j][:, w0 : w0 + KW],
                            wv_sb[j][:],
                            start=(j == 0),
                            stop=False,
                        )
                    # +bv via ones-row rank-1 update
                    nc.tensor.matmul(
                        ps[:KW, :],
                        ones_row[:],
                        bv_sb[:],
                        start=False,
                        stop=True,
                    )
                    nc.scalar.copy(v_sb[:, pt * C : (pt + 1) * C], ps[:KW, :])

                # ---- lo phase: 7 patch-tiles of 448 q tokens ----
                for pt in range(NPT):
                    t0 = b * P * NQ + pt * QW  # global token offset in loT
                    loT_sb = [
                        lopool.tile([128, QW], BF16, tag=f"loT{j}", name=f"loT{j}")
                        for j in range(NCH)
                    ]
                    for j in range(NCH):
                        nc.sync.dma_start(
                            loT_sb[j][:],
                            loT_d[128 * j : 128 * (j + 1), t0 : t0 + QW],
                        )
                    # Q^T by c-out chunk
                    qT_sb = [
                        lopool.tile([128, QW], BF16, tag=f"qT{m}", name=f"qT{m}")
                        for m in range(NCH)
                    ]
                    for m in range(NCH):
                        ps = pproj.tile([128, QW], F32, tag="proj", name="proj")
                        for j in range(NCH):
                            nc.tensor.matmul(
                                ps[:],
                                wq_sb[j][:, 128 * m : 128 * (m + 1)],
                                loT_sb[j][:],
                                start=(j == 0),
                                stop=(j == NCH - 1),
                            )
                        nc.vector.tensor_scalar_add(
                            qT_sb[m][:], ps[:], bq_sb[:, m : m + 1]
                        )
                    # gate partial sums on DVE: gsum[c_in%128, tok]
                    gsum = wpool.tile([128, QW], BF16, tag="gsum", name="gsum")
                    nc.vector.tensor_scalar_mul(
                        gsum[:], loT_sb[0][:], ws_sb[:, 0:1]
                    )
                    for j in range(1, NCH):
                        nc.vector.scalar_tensor_tensor(
                            gsum[:],
                            loT_sb[j][:],
                            ws_sb[:, j : j + 1],
                            gsum[:],
                            op0=mybir.AluOpType.mult,
                            op1=mybir.AluOpType.add,
                        )
                    # scores^T [112, 448] dense block
                    sc = psc.tile([KW, QW], F32, tag="sc", name="sc")
                    for j in range(NCH):
                        nc.tensor.matmul(
                            sc[:],
                            kT_sb[j][:, pt * KW : (pt + 1) * KW],
                            qT_sb[j][:],
                            start=(j == 0),
                            stop=(j == NCH - 1),
                        )
                    expT = wpool.tile([KW, QW], BF16, tag="expT", name="expT")
                    nc.scalar.activation(
                        expT[:], sc[:], mybir.ActivationFunctionType.Exp,
                        scale=SCALE,
                    )
                    expTm = wpool.tile([KW, QW], BF16, tag="expTm", name="expTm")
                    nc.vector.tensor_mul(expTm[:], expT[:], mask_sb[:])

                    # denom + gate matmuls into one psum tile [128, 8]
                    gd = pgd.tile([128, 2 * NCH], F32, tag="gd", name="gd")
                    for g in range(4):
                        kk = 32 if g < 3 else 16
                        mm = 128 if g < 3 else 64
                        nc.tensor.matmul(
                            gd[:mm, g : g + 1],
                            expTm[32 * g : 32 * g + kk, 128 * g : 128 * g + mm],
                            ones_col[32 * g : 32 * g + kk, :],
                            tile_position=(32 * g, 0),
                        )
                        nc.tensor.matmul(
                            gd[:mm, 4 + g : 5 + g],
                            gsum[:, 128 * g : 128 * g + mm],
                            ones_col[:, :],
                        )
                    # s = gelu(gate + bs) / denom  [128, 4] f32
                    gatev = wpool.tile([128, NCH], F32, tag="gatev", name="gatev")
                    rec = wpool.tile([128, NCH], F32, tag="rec", name="rec")
                    s_sb = wpool.tile([128, NCH], F32, tag="s", name="s")
                    nc.scalar.activation(
                        gatev[:, 0:3], gd[:, 4:7], act_fn, bias=bs_sb[:],
                    )
                    nc.scalar.activation(
                        gatev[:64, 3:4], gd[:64, 7:8], act_fn,
                        bias=bs_sb[:64, :],
                    )
                    nc.vector.reciprocal(rec[:, 0:3], gd[:, 0:3])
                    nc.vector.reciprocal(rec[:64, 3:4], gd[:64, 3:4])
                    nc.vector.tensor_mul(s_sb[:, 0:3], gatev[:, 0:3], rec[:, 0:3])
                    nc.vector.tensor_mul(
                        s_sb[:64, 3:4], gatev[:64, 3:4], rec[:64, 3:4]
                    )

                    # attn @ v in 2-patch groups, scale, store
                    for g in range(4):
                        kk = 32 if g < 3 else 16
                        mm = 128 if g < 3 else 64
                        ao = pvao.tile([128, C], F32, tag="vao", name="vao")
                        nc.tensor.matmul(
                            ao[:mm, :],
                            expTm[32 * g : 32 * g + kk, 128 * g : 128 * g + mm],
                            v_sb[32 * g : 32 * g + kk, pt * C : (pt + 1) * C],
                            tile_position=(32 * g, 0),
                        )
                        aout = apool.tile([128, C], F32, tag="aout", name="aout")
                        nc.scalar.activation(
                            aout[:mm, :], ao[:mm, :],
                            mybir.ActivationFunctionType.Copy,
                            scale=s_sb[:mm, g : g + 1],
                        )
                        r0 = t0 + 128 * g
                        nc.sync.dma_start(out_d[r0 : r0 + mm, :], aout[:mm, :])

            if repeat == 1:
                body()
            else:
                with tc.For_i(0, repeat, 1):
                    body()

    nc.compile()
    return nc


def _patch(x, hw, k):
    b = x.shape[0]
    c = x.shape[-1]
    g = hw // k
    x = x.reshape(b, g, k, g, k, c).transpose(0, 1, 3, 2, 4, 5)
    return x.reshape(b, g * g, k * k, c)


def _unpatch(x, hw, k):
    b, p, n, c = x.shape
    g = hw // k
    x = x.reshape(b, g, g, k, k, c).transpose(0, 1, 3, 2, 4, 5)
    return x.reshape(b, hw * hw, c)


def _host_prep(lo, hi, Wq, bq, Wk, bk, Wv, bv, Ws, bs):
    """Build per-core in_maps. Returns (in_maps, lo_p) with lo_p fp32
    [B, P, NQ, C] kept for the host-side residual."""
    lo_p = _patch(np.asarray(lo, np.float32), LOHW, RATE)   # [B,49,64,C]
    hi_p = _patch(np.asarray(hi, np.float32), HIHW, BASE)   # [B,49,16,C]

    wqT = np.ascontiguousarray(np.asarray(Wq, np.float32).T).astype(NPBF16)
    wkT = np.ascontiguousarray(np.asarray(Wk, np.float32).T).astype(NPBF16)
    wvT = np.ascontiguousarray(np.asarray(Wv, np.float32).T).astype(NPBF16)
    ws2 = np.ascontiguousarray(
        np.asarray(Ws, np.float32).reshape(NCH, 128).T
    ).astype(np.float32)
    bq2 = np.ascontiguousarray(np.asarray(bq, np.float32).reshape(NCH, 128).T)
    bk2 = np.ascontiguousarray(np.asarray(bk, np.float32).reshape(NCH, 128).T)
    bv2 = np.asarray(bv, np.float32).reshape(1, C).astype(NPBF16)
    bs2 = np.full((128, 1), float(np.asarray(bs).reshape(-1)[0]), np.float32)

    mask = np.zeros((KW, QW), np.float32)
    for p in range(PT):
        mask[NK * p : NK * (p + 1), NQ * p : NQ * (p + 1)] = 1.0
    mask = mask.astype(NPBF16)

    in_maps = []
    for cid in range(N_CORES):
        bs_lo = lo_p[NB * cid : NB * (cid + 1)].reshape(TLO, C)
        bs_hi = hi_p[NB * cid : NB * (cid + 1)].reshape(THI, C)
        loT = np.ascontiguousarray(bs_lo.T).astype(NPBF16)
        hiT = np.ascontiguousarray(bs_hi.T).astype(NPBF16)
        in_maps.append(
            dict(
                loT=loT, hiT=hiT, wqT=wqT, wkT=wkT, wvT=wvT,
                ws=ws2, bq=bq2, bk=bk2, bv=bv2, bs=bs2, mask=mask,
            )
        )
    return in_maps, lo_p


_PROG_CACHE = {}


def _get_program():
    if "nc" not in _PROG_CACHE:
        _PROG_CACHE["nc"] = build_program()
    return _PROG_CACHE["nc"]


def kernel(lo, hi, Wq, bq, Wk, bk, Wv, bv, Ws, bs, lohw, hihw):
    assert int(lohw) == LOHW and int(hihw) == HIHW
    in_maps, lo_p = _host_prep(lo, hi, Wq, bq, Wk, bk, Wv, bv, Ws, bs)
    nc = _get_program()
    res = run_bass_kernel_spmd(nc, in_maps, core_ids=list(range(N_CORES)))
    gated = np.concatenate(
        [res.results[cid]["out"] for cid in range(N_CORES)], axis=0
    ).reshape(B, P, NQ, C)
    out_p = gated.astype(np.float32) + lo_p
    return _unpatch(out_p, LOHW, RATE).astype(np.float32)


if __name__ == "__main__":
    nc = build_program()
    print("program built ok")


# revision 13
# speedup vs baseline: 1.8421x; 1.8421x over previous
"""Patch-local cross attention (CSA) TRN2 kernel.

Problem (hardcoded shapes): B=32, C=512, lohw=56, hihw=28.
base = hihw//7 = 4, rate = (lohw//hihw)*base = 8.
lo_p: [B, 49, 64, C], hi_p: [B, 49, 16, C] (7x7 patch grid).
q = lo_p@WqT+bq; k = hi_p@WkT+bk; v = hi_p@WvT+bv
gate = gelu(lo_p@Ws.T+bs)  (exact erf gelu)
out = softmax(q k^T / sqrt(C)) @ v * gate + lo_p, unpatched.

Sharding: data-parallel over B across 8 cores (4 batch items each).
Host does patching/transposes/bf16 casts and the final fp32 residual
add (+lo_p) + unpatch; device does all matmuls, softmax, gelu, scaling.

Device structure per core (NB=4 batch items), per batch item b:
  KV phase: K^T [c, 784] and V [112, 7*512] resident in SBUF (bf16).
  Phase 1, per patch-tile pt (7 patches = 448 q tok, 112 kv tok):
    Q^T chunks (PE, fp32 psum -> bf16 sbuf via DVE bias-add),
    dense scores^T [112,448] (PE), exp (ACT), block-diag mask (DVE),
    gate partials (DVE); denom+gate rank-1 matmuls -> gd psum [128,8]
    (pre-memset 1.0), staged to sbuf (DVE).
  Phase 2 (once per b): gelu(gate)+bias (ACT), recip(denom) (DVE),
    s = gelu*recip [128, 28] (DVE).
  Phase 3, per pt: unnormalized attn@v in 2-patch groups (K=32, PE),
    scale by s per-token (DVE), store (HWDGE on ACT ring).
Engine discipline (measured on this fabric): loads on nc.sync ring,
stores on nc.scalar ring (mixing directions on one HWDGE ring
serializes ~3us/DMA); ACT runs only exp + one gelu per b (ACT ops pay
~1.8us table-reload costs); everything else DVE.
Output: gated attention term [NB*3136, 512] fp32 (residual on host).
"""

import sys

if "/opt/trn_rl_repo" not in sys.path:
    sys.path.insert(0, "/opt/trn_rl_repo")

import numpy as np
import ml_dtypes

import concourse.bacc as bacc
import concourse.bass as bass
import concourse.mybir as mybir
from concourse import tile
from concourse.bass_utils import run_bass_kernel_spmd

BF16 = mybir.dt.bfloat16
F32 = mybir.dt.float32
NPBF16 = ml_dtypes.bfloat16

N_CORES = 8
B, C, LOHW, HIHW = 32, 512, 56, 28
RATE, BASE = 8, 4
G = 7               # patch grid side
P = G * G           # 49 patches
NQ = RATE * RATE    # 64 q tokens / patch
NK = BASE * BASE    # 16 kv tokens / patch
NB = B // N_CORES   # batch items per core
TLO = NB * P * NQ   # 12544 lo tokens per core
THI = NB * P * NK   # 3136 hi tokens per core
NCH = C // 128      # 4 contraction chunks
PT = 7              # patches per tile
QW = PT * NQ        # 448 q tokens per patch-tile
KW = PT * NK        # 112 kv tokens per patch-tile
NPT = P // PT       # 7 patch-tiles per batch item
SCALE = float(C) ** -0.5


def build_program(n_b: int = NB, gate_act: str = "gelu", repeat: int = 1,
                  store_ring: str = "sync", zero_bias: bool = True):
    """Emit the Bass/Tile program for one core processing n_b batch items.

    gate_act="identity" substitutes the gate's Gelu with Identity so the
    program can run under CoreSim (which lacks a Gelu model).
    repeat>1 wraps the whole body in a hardware For_i loop (for timing)."""
    act_fn = (
        mybir.ActivationFunctionType.Gelu
        if gate_act == "gelu"
        else mybir.ActivationFunctionType.Identity
    )
    store_eng_name = store_ring
    nc = bacc.Bacc(
        "TRN2",
        target_bir_lowering=False,
        debug=False,
        num_devices=N_CORES,
    )
    tlo = n_b * P * NQ
    thi = n_b * P * NK

    loT_d = nc.dram_tensor("loT", [C, tlo], BF16, kind="ExternalInput").ap()
    hiT_d = nc.dram_tensor("hiT", [C, thi], BF16, kind="ExternalInput").ap()
    wqT_d = nc.dram_tensor("wqT", [C, C], BF16, kind="ExternalInput").ap()
    wkT_d = nc.dram_tensor("wkT", [C, C], BF16, kind="ExternalInput").ap()
    wvT_d = nc.dram_tensor("wvT", [C, C], BF16, kind="ExternalInput").ap()
    ws_d = nc.dram_tensor("ws", [128, NCH], F32, kind="ExternalInput").ap()
    bq_d = nc.dram_tensor("bq", [128, NCH], F32, kind="ExternalInput").ap()
    bk_d = nc.dram_tensor("bk", [128, NCH], F32, kind="ExternalInput").ap()
    bv_d = nc.dram_tensor("bv", [1, C], BF16, kind="ExternalInput").ap()
    bs_d = nc.dram_tensor("bs", [128, 1], F32, kind="ExternalInput").ap()
    mask_d = nc.dram_tensor("mask", [KW, QW], BF16, kind="ExternalInput").ap()
    out_d = nc.dram_tensor("out", [tlo, C], F32, kind="ExternalOutput").ap()

    store_eng = nc.scalar if store_eng_name == "act" else nc.sync
    with tile.TileContext(nc) as tc:
        with (
            tc.tile_pool(name="const", bufs=1) as cpool,
            tc.tile_pool(name="kv", bufs=2) as kvpool,
            tc.tile_pool(name="work", bufs=2) as wpool,
            tc.tile_pool(name="lo", bufs=2) as lopool,
            tc.tile_pool(name="aout", bufs=4) as apool,
            tc.tile_pool(name="pproj", bufs=2, space=bass.MemorySpace.PSUM) as pproj,
            tc.tile_pool(name="pvao", bufs=2, space=bass.MemorySpace.PSUM) as pvao,
            tc.tile_pool(name="psc", bufs=2, space=bass.MemorySpace.PSUM) as psc,
            tc.tile_pool(name="pgd", bufs=2, space=bass.MemorySpace.PSUM) as pgd,
        ):
            # ---- constants ----
            wq_sb = [cpool.tile([128, C], BF16, tag=f"wq{j}", name=f"wq{j}")
                     for j in range(NCH)]
            wk_sb = [cpool.tile([128, C], BF16, tag=f"wk{j}", name=f"wk{j}")
                     for j in range(NCH)]
            wv_sb = [cpool.tile([128, C], BF16, tag=f"wv{j}", name=f"wv{j}")
                     for j in range(NCH)]
            for j in range(NCH):
                nc.sync.dma_start(wq_sb[j][:], wqT_d[128 * j : 128 * (j + 1), :])
                nc.sync.dma_start(wk_sb[j][:], wkT_d[128 * j : 128 * (j + 1), :])
                nc.sync.dma_start(wv_sb[j][:], wvT_d[128 * j : 128 * (j + 1), :])
            ws_sb = cpool.tile([128, NCH], F32, tag="ws", name="ws")
            bq_sb = cpool.tile([128, NCH], F32, tag="bq", name="bq")
            bk_sb = cpool.tile([128, NCH], F32, tag="bk", name="bk")
            bv_sb = cpool.tile([1, C], BF16, tag="bv", name="bv")
            bs_sb = cpool.tile([128, 1], F32, tag="bs", name="bs")
            mask_sb = cpool.tile([KW, QW], BF16, tag="mask", name="mask")
            nc.sync.dma_start(ws_sb[:], ws_d[:])
            nc.sync.dma_start(bq_sb[:], bq_d[:])
            nc.sync.dma_start(bk_sb[:], bk_d[:])
            nc.sync.dma_start(bv_sb[:], bv_d[:])
            nc.sync.dma_start(bs_sb[:], bs_d[:])
            nc.sync.dma_start(mask_sb[:], mask_d[:])
            ones_col = cpool.tile([128, 1], BF16, tag="ones_col", name="ones_col")
            ones_row = cpool.tile([1, KW], BF16, tag="ones_row", name="ones_row")
            nc.vector.memset(ones_col[:], 1.0)
            nc.vector.memset(ones_row[:], 1.0)

            def body():
              for b in range(n_b):
                # ---- K/V phase ----
                hiT_sb = kvpool.tile([128, NCH, P * NK], BF16, tag="hiT",
                                     name="hiT")
                nc.sync.dma_start(
                    hiT_sb[:],
                    hiT_d.rearrange("(j p) t -> p j t", p=128)[
                        :, :, b * P * NK : (b + 1) * P * NK],
                )
                kT_sb = [
                    kvpool.tile([128, P * NK], BF16, tag=f"kT{j}",
                                name=f"kT{j}")
                    for j in range(NCH)
                ]
                halves = [(0, 448), (448, P * NK)]
                for m in range(NCH):
                    for h0, h1 in halves:
                        ps = pproj.tile([128, QW], F32, tag="proj", name="proj")
                        for j in range(NCH):
                            nc.tensor.matmul(
                                ps[:, : h1 - h0],
                                wk_sb[j][:, 128 * m : 128 * (m + 1)],
                                hiT_sb[:, j, h0:h1],
                                start=(j == 0),
                                stop=(j == NCH - 1),
                            )
                        if zero_bias:
                            nc.vector.tensor_copy(
                                kT_sb[m][:, h0:h1], ps[:, : h1 - h0]
                            )
                        else:
                            nc.vector.tensor_scalar_add(
                                kT_sb[m][:, h0:h1],
                                ps[:, : h1 - h0],
                                bk_sb[:, m : m + 1],
                            )
                v_sb = kvpool.tile([KW, NPT * C], BF16, tag="v", name="v")
                for pt in range(NPT):
                    w0 = pt * KW
                    ps = pvao.tile([128, C], F32, tag="vao", name="vao")
                    for j in range(NCH):
                        nc.tensor.matmul(
                            ps[:KW, :],
                            hiT_sb[:, j, w0 : w0 + KW],
                            wv_sb[j][:],
                            start=(j == 0),
                            stop=False,
                        )
                    nc.tensor.matmul(
                        ps[:KW, :], ones_row[:], bv_sb[:],
                        start=False, stop=True,
                    )
                    nc.vector.tensor_copy(
                        v_sb[:, pt * C : (pt + 1) * C], ps[:KW, :]
                    )

                # per-b staging for denom/gate/scale [128, 4*NPT]
                den_st = wpool.tile([128, 4 * NPT], F32, tag="den", name="den")
                gate_st = wpool.tile([128, 4 * NPT], F32, tag="gate",
                                     name="gate")
                s_st = wpool.tile([128, 4 * NPT], F32, tag="s", name="s")
                expTm_all = []

                # ---- phase 1: per patch-tile ----
                for pt in range(NPT):
                    t0 = b * P * NQ + pt * QW
                    loT_sb = lopool.tile([128, NCH, QW], BF16, tag="loT",
                                         name="loT", bufs=3)
                    nc.sync.dma_start(
                        loT_sb[:],
                        loT_d.rearrange("(j p) t -> p j t", p=128)[
                            :, :, t0 : t0 + QW],
                    )
                    qT_sb = [
                        lopool.tile([128, QW], BF16, tag=f"qT{m}",
                                    name=f"qT{m}")
                        for m in range(NCH)
                    ]
                    for m in range(NCH):
                        ps = pproj.tile([128, QW], F32, tag="proj", name="proj")
                        for j in range(NCH):
                            nc.tensor.matmul(
                                ps[:],
                                wq_sb[j][:, 128 * m : 128 * (m + 1)],
                                loT_sb[:, j, :],
                                start=(j == 0),
                                stop=(j == NCH - 1),
                            )
                        if zero_bias:
                            nc.vector.tensor_copy(qT_sb[m][:], ps[:])
                        else:
                            nc.vector.tensor_scalar_add(
                                qT_sb[m][:], ps[:], bq_sb[:, m : m + 1]
                            )
                    gsum = wpool.tile([128, QW], BF16, tag="gsum", name="gsum")
                    nc.vector.tensor_scalar_mul(
                        gsum[:], loT_sb[:, 0, :], ws_sb[:, 0:1]
                    )
                    for j in range(1, NCH):
                        nc.vector.scalar_tensor_tensor(
                            gsum[:],
                            loT_sb[:, j, :],
                            ws_sb[:, j : j + 1],
                            gsum[:],
                            op0=mybir.AluOpType.mult,
                            op1=mybir.AluOpType.add,
                        )
                    sc = psc.tile([KW, QW], F32, tag="sc", name="sc")
                    for j in range(NCH):
                        nc.tensor.matmul(
                            sc[:],
                            kT_sb[j][:, pt * KW : (pt + 1) * KW],
                            qT_sb[j][:],
                            start=(j == 0),
                            stop=(j == NCH - 1),
                        )
                    expT = wpool.tile([KW, QW], BF16, tag="expT", name="expT")
                    nc.scalar.activation(
                        expT[:], sc[:], mybir.ActivationFunctionType.Exp,
                        scale=SCALE,
                    )
                    expTm = wpool.tile([KW, QW], BF16, tag="expTm",
                                       name="expTm", bufs=NPT + 4)
                    nc.vector.tensor_mul(expTm[:], expT[:], mask_sb[:])
                    expTm_all.append(expTm)

                    gd = pgd.tile([128, 2 * NCH], F32, tag="gd", name="gd")
                    nc.vector.memset(gd[:], 1.0)
                    for g in range(4):
                        kk = 32 if g < 3 else 16
                        mm = 128 if g < 3 else 64
                        nc.tensor.matmul(
                            gd[:mm, g : g + 1],
                            expTm[32 * g : 32 * g + kk, 128 * g : 128 * g + mm],
                            ones_col[32 * g : 32 * g + kk, :],
                            tile_position=(32 * g, 0),
                        )
                        nc.tensor.matmul(
                            gd[:mm, 4 + g : 5 + g],
                            gsum[:, 128 * g : 128 * g + mm],
                            ones_col[:, :],
                        )
                    nc.vector.tensor_copy(
                        den_st[:, 4 * pt : 4 * pt + 4], gd[:, 0:4]
                    )
                    nc.vector.tensor_copy(
                        gate_st[:, 4 * pt : 4 * pt + 4], gd[:, 4:8]
                    )

                # ---- phase 2: one gelu + recip + mul per b ----
                gatev = wpool.tile([128, 4 * NPT], F32, tag="gatev",
                                   name="gatev")
                nc.scalar.activation(gatev[:], gate_st[:], act_fn,
                                     bias=bs_sb[:])
                rec = wpool.tile([128, 4 * NPT], F32, tag="rec", name="rec")
                nc.vector.reciprocal(rec[:], den_st[:])
                nc.vector.tensor_mul(s_st[:], gatev[:], rec[:])

                # ---- phase 3: attn@v, scale, store ----
                for pt in range(NPT):
                    t0 = b * P * NQ + pt * QW
                    expTm = expTm_all[pt]
                    aout = apool.tile([128, 4, C], F32, tag="aout",
                                      name="aout", bufs=3)
                    for g in range(4):
                        kk = 32 if g < 3 else 16
                        mm = 128 if g < 3 else 64
                        ao = pvao.tile([128, C], F32, tag="vao", name="vao")
                        nc.tensor.matmul(
                            ao[:mm, :],
                            expTm[32 * g : 32 * g + kk, 128 * g : 128 * g + mm],
                            v_sb[32 * g : 32 * g + kk, pt * C : (pt + 1) * C],
                            tile_position=(32 * g, 0),
                        )
                        nc.vector.tensor_scalar_mul(
                            aout[:mm, g, :], ao[:mm, :],
                            s_st[:mm, 4 * pt + g : 4 * pt + g + 1],
                        )
                    # one store for groups 0-2 (384 rows), one for group 3
                    store_eng.dma_start(
                        out_d[t0 : t0 + 384, :].rearrange(
                            "(a p) c -> p a c", p=128),
                        aout[:, 0:3, :],
                    )
                    store_eng.dma_start(
                        out_d[t0 + 384 : t0 + 448, :], aout[:64, 3, :]
                    )

            if repeat == 1:
                body()
            else:
                with tc.For_i(0, repeat, 1):
                    body()

    nc.compile()
    return nc


def _patch(x, hw, k):
    b = x.shape[0]
    c = x.shape[-1]
    g = hw // k
    x = x.reshape(b, g, k, g, k, c).transpose(0, 1, 3, 2, 4, 5)
    return x.reshape(b, g * g, k * k, c)


def _unpatch(x, hw, k):
    b, p, n, c = x.shape
    g = hw // k
    x = x.reshape(b, g, g, k, k, c).transpose(0, 1, 3, 2, 4, 5)
    return x.reshape(b, hw * hw, c)


def _host_prep(lo, hi, Wq, bq, Wk, bk, Wv, bv, Ws, bs):
    """Build per-core in_maps. Returns (in_maps, lo_p) with lo_p fp32
    [B, P, NQ, C] kept for the host-side residual."""
    lo_p = _patch(np.asarray(lo, np.float32), LOHW, RATE)   # [B,49,64,C]
    hi_p = _patch(np.asarray(hi, np.float32), HIHW, BASE)   # [B,49,16,C]

    wqT = np.ascontiguousarray(np.asarray(Wq, np.float32).T).astype(NPBF16)
    wkT = np.ascontiguousarray(np.asarray(Wk, np.float32).T).astype(NPBF16)
    wvT = np.ascontiguousarray(np.asarray(Wv, np.float32).T).astype(NPBF16)
    ws2 = np.ascontiguousarray(
        np.asarray(Ws, np.float32).reshape(NCH, 128).T
    ).astype(np.float32)
    bq2 = np.ascontiguousarray(np.asarray(bq, np.float32).reshape(NCH, 128).T)
    bk2 = np.ascontiguousarray(np.asarray(bk, np.float32).reshape(NCH, 128).T)
    bv2 = np.asarray(bv, np.float32).reshape(1, C).astype(NPBF16)
    bs2 = np.full((128, 1), float(np.asarray(bs).reshape(-1)[0]), np.float32)

    mask = np.zeros((KW, QW), np.float32)
    for p in range(PT):
        mask[NK * p : NK * (p + 1), NQ * p : NQ * (p + 1)] = 1.0
    mask = mask.astype(NPBF16)

    in_maps = []
    for cid in range(N_CORES):
        bs_lo = lo_p[NB * cid : NB * (cid + 1)].reshape(TLO, C)
        bs_hi = hi_p[NB * cid : NB * (cid + 1)].reshape(THI, C)
        loT = np.ascontiguousarray(bs_lo.T).astype(NPBF16)
        hiT = np.ascontiguousarray(bs_hi.T).astype(NPBF16)
        in_maps.append(
            dict(
                loT=loT, hiT=hiT, wqT=wqT, wkT=wkT, wvT=wvT,
                ws=ws2, bq=bq2, bk=bk2, bv=bv2, bs=bs2, mask=mask,
            )
        )
    return in_maps, lo_p


_PROG_CACHE = {}


def _get_program(zero_bias: bool):
    key = ("nc", zero_bias)
    if key not in _PROG_CACHE:
        _PROG_CACHE[key] = build_program(zero_bias=zero_bias)
    return _PROG_CACHE[key]


def kernel(lo, hi, Wq, bq, Wk, bk, Wv, bv, Ws, bs, lohw, hihw):
    assert int(lohw) == LOHW and int(hihw) == HIHW
    in_maps, lo_p = _host_prep(lo, hi, Wq, bq, Wk, bk, Wv, bv, Ws, bs)
    zb = (not np.any(np.asarray(bq))) and (not np.any(np.asarray(bk)))
    nc = _get_program(zero_bias=bool(zb))
    res = run_bass_kernel_spmd(nc, in_maps, core_ids=list(range(N_CORES)))
    gated = np.concatenate(
        [res.results[cid]["out"] for cid in range(N_CORES)], axis=0
    ).reshape(B, P, NQ, C)
    out_p = gated.astype(np.float32) + lo_p
    return _unpatch(out_p, LOHW, RATE).astype(np.float32)


if __name__ == "__main__":
    nc = build_program()
    print("program built ok")
